# revision 14
# baseline (speedup 1.0000x reference)
"""Trainium2 Bass kernel for the ArcModel3Phase loss (y-sorted redesign).

Math: per point m, logmix = ln(sum_j e^{l_j}) over 6 mixture components
(3 interior Gaussians + 3 MC-integrated interface terms of N=1024 samples
each).  Writing l = A(x,y) + h with A = lny - x^2/2sn^2 - y^2/sn^2 and h
affine in (x, y, lny, 1), every component (and the per-m bias) becomes a
column of ONE bf16 matmul over 17 lhsT rows:

  R[p, c] = sum_k lhsT[k, p] rhs[k, c]   -> exp -> segmented row sums.

Device work per point is ~100 columns instead of 3072 thanks to:
  1. Global y-sort (host permutes; the loss is a sum over m, so no
     unpermute).  Each 1024-point block has a narrow y-range, so most MC
     samples are irrelevant to it: a sample contributes only within
     |y - G(tx)| ~ 0.2.  Host prunes per block against a logmix lower
     bound on an x-grid (cutoff e^-PRUNE).
  2. Adaptive sample merging (2nd-order cumulant, exact variance carried
     as 5 extra matmul rows) with a per-block relevance window, plus an
     overshoot guard that keeps each merged column within OCAP of the
     exact logsumexp at window probes (prevents f32 exp overflow and
     bounds the merge error).
  3. The e^{R2} subtraction pass (Bessel 1-e^{-w} expansion) is skipped
     for samples with w = 4yG/sn^2 >= WSKIP for the whole block - almost
     all of them once y is sorted.
  4. The per-m exp bias nu = b - A (b = max of per-component upper
     bounds, a tight cover of max_j l_j) is pure host math, folded into
     the matmul as two hi/lo bf16 rows.  No on-device max pass at all.
  5. Interior components are affine in (x, lny): 3 more columns, two
     lny rows.  The final ln + masked sum runs on host from the DMA'd
     [128, T] mix tile (f64, more accurate than device f32 accum).

One EXP instruction covers a whole batch of tiles (PSUM budget 2048
f32), then two segmented DVE reduces produce S1 (R1+interior) and S2
per tile; mix = S1 - S2.
"""
import math

import numpy as np
import ml_dtypes
from scipy.special import erf, erfinv

import concourse.bass as bass
import concourse.tile as tile
from concourse import bacc, mybir
from concourse.bass_utils import run_bass_kernel_spmd

BF16 = ml_dtypes.bfloat16
WF = 3.0
LOG2PI = math.log(2.0 * math.pi)
M = 100_000
N_MC = 1024
P = 128
N_CORES = 8
BLK = P * N_CORES              # 1024 points per global block
T = (M + BLK - 1) // BLK       # 98 tiles per core
M_PAD = T * BLK
ROWS = 15
DEAD_B = -30000.0

DM = 24.0                      # max in-window |h - mean| within a group
KMAX = 96                      # max group size
PRUNE = 7.0                    # per-block relevance cutoff (e-folds)
WSKIP = 9.0                    # skip R2 columns with w >= this block-wide
OCAP = 3.5                     # max merged-vs-exact LSE overshoot
XWIN = 0.40                    # merge relevance half-window in x
PSUM_BUDGET = 2048             # f32 columns per batch (4 PSUM banks)
BMAX = 16                      # max tiles per batch

_graph_cache = {}
_last_results = None


def _split(a):
    hi = np.asarray(a).astype(BF16)
    lo = (np.asarray(a, np.float64) - hi.astype(np.float64)).astype(BF16)
    return hi, lo


def _host_rows(ku, Ia, Ib, sigma_b, sigma_n, logw):
    """Raw per-sample rows for one interface term (float64, tx-sorted)."""
    ku = np.asarray(ku, np.float64)
    sn2 = sigma_n ** 2
    I_min = Ia + 0.5 * (Ib - Ia) * (1.0 + erf(-WF / np.sqrt(2.0)))
    I_diff = (Ib - Ia) * erf(WF / np.sqrt(2.0))
    tx = np.sort(ku * I_diff + I_min)
    ei = erfinv(2.0 * (tx - Ia) / (Ib - Ia) - 1.0)
    G = (Ib - Ia) / np.sqrt(2.0 * np.pi * sigma_b ** 2) * np.exp(-ei ** 2)
    lptx = -np.log(2.0 * WF * (Ib - Ia)) + 0.5 * LOG2PI + ei ** 2
    B = -0.5 * tx ** 2 / sn2 - np.log(G) - G ** 2 / sn2 + lptx
    C0 = (-np.log(sigma_n) - 0.5 * LOG2PI
          + np.log(2.0) - 2.0 * np.log(sigma_n)
          + 0.5 * np.log(2.0 / np.pi) - np.log(2.0)
          - 0.5 * np.log(2.0) + np.log(sigma_n))
    Bp = B + np.log(I_diff) - np.log(N_MC) + logw + C0
    return tx, tx / sn2, 2.0 * G / sn2, Bp, G


def _raw_l(xg, yv, term, sn2):
    """l_n(xg, yv) for all samples of one term: [X, N]."""
    tx, txp, g1, Bp, G = term
    w = np.minimum(4.0 * yv * G / sn2, 700.0)
    return (Bp[None, :] + xg[:, None] * txp[None, :] + yv * g1[None, :]
            + np.log1p(-np.exp(-w))[None, :]
            + np.log(yv) - 0.5 * (xg[:, None] ** 2) / sn2 - yv * yv / sn2)


def _interior_logp(x, y, I, sn):
    return (math.log(2.0) + 2.0 * np.log(y) - math.lgamma(1.5)
            - 3.0 * math.log(sn) - (y / sn) ** 2
            - math.log(sn) - 0.5 * LOG2PI - 0.5 * ((x - I) / sn) ** 2)


def _plan(x, y, ku12, ku23, ku13, sigma_b, sigma_n, I1, I2, I3, w):
    x = np.asarray(x, np.float64)
    y = np.asarray(y, np.float64)
    sn = float(sigma_n); sb = float(sigma_b)
    I1, I2, I3 = float(I1), float(I2), float(I3)
    w64 = np.asarray(w, np.float64)
    logw = w64 - (np.log(np.sum(np.exp(w64 - w64.max()))) + w64.max())
    sn2 = sn * sn

    terms = [_host_rows(ku, Ia, Ib, sb, sn, float(logw[3 + j]))
             for j, (ku, Ia, Ib) in enumerate(
                 ((ku12, I1, I2), (ku23, I2, I3), (ku13, I1, I3)))]

    # l(x,y) <= lny + c_u: per-sample peak at (tx, G), minus its lny part
    c_u = -1e30
    for tx, txp, g1, Bp, G in terms:
        l_peak = (np.log(G) + 0.5 * tx ** 2 / sn2 + G ** 2 / sn2 + Bp
                  + np.log1p(-np.exp(-np.minimum(4.0 * G * G / sn2, 700.0))))
        c_u = max(c_u, float((l_peak - np.log(G)).max()))

    order = np.argsort(y, kind="stable")
    pad = M_PAD - len(x)
    order_p = np.concatenate([order, np.repeat(order[-1], pad)])
    mask_p = np.concatenate([np.ones(len(x), np.float32),
                             np.zeros(pad, np.float32)])
    ys = y[order_p]

    xmin, xmax = float(x.min()), float(x.max())
    xg = np.linspace(xmin, xmax, 121)

    def logmix_lb(yv):
        mx = np.maximum.reduce([_interior_logp(xg, yv, I, sn) + logw[k]
                                for k, I in enumerate((I1, I2, I3))])
        for term in terms:
            l = _raw_l(xg, yv, term, sn2)
            m2 = l.max(axis=1)
            mx = np.maximum(mx, m2 + np.log(
                np.sum(np.exp(l - m2[:, None]), axis=1)))
        return mx

    tiles = []
    for t in range(T):
        blk = slice(t * BLK, (t + 1) * BLK)
        yb = ys[blk]
        ylo, yhi = float(yb.min()), float(yb.max())
        yprobes = np.linspace(ylo, yhi, 3)
        lmix = np.max([logmix_lb(yv) for yv in yprobes], axis=0)

        tile_terms = []
        for term in terms:
            tx, txp, g1, Bp, G = term
            keep = np.zeros(len(tx), bool)
            for yv in yprobes:
                l = _raw_l(xg, yv, term, sn2)
                keep |= (l - lmix[:, None] >= -PRUNE).any(axis=0)
            yc = np.clip(G, ylo, yhi)
            wv = np.minimum(4.0 * yc * G / sn2, 700.0)
            l = (Bp[None, :] + xg[:, None] * txp[None, :]
                 + (yc * g1)[None, :] + np.log1p(-np.exp(-wv))[None, :]
                 + np.log(yc)[None, :] - 0.5 * (xg[:, None] ** 2) / sn2
                 - (yc * yc)[None, :] / sn2)
            keep |= (l - lmix[:, None] >= -PRUNE).any(axis=0)

            idx = np.where(keep)[0]
            if len(idx) == 0:
                tile_terms.append(None)
                continue
            txk, txpk, g1k, Bpk, Gk = (tx[idx], txp[idx], g1[idx],
                                       Bp[idx], G[idx])
            groups = []
            i = 0
            n = len(idx)
            while i < n:
                k = min(KMAX, n - i)
                while k > 1:
                    tt, gg, bb = txpk[i:i + k], g1k[i:i + k], Bpk[i:i + k]
                    txm = txk[i:i + k].mean()
                    lo = max(xmin, txm - XWIN); hi = min(xmax, txm + XWIN)
                    dt = tt - tt.mean(); db = bb - bb.mean()
                    dg = np.abs(gg - gg.mean())
                    d = (np.maximum(np.abs(dt * lo + db),
                                    np.abs(dt * hi + db)) + dg * yhi)
                    if d.max() <= DM:
                        xp = np.array([lo, 0.5 * (lo + hi), hi])
                        yp = np.array([ylo, yhi])
                        ok = True
                        for sgn in (1.0, -1.0):
                            h = (xp[:, None, None] * tt[None, None, :]
                                 + sgn * yp[None, :, None] * gg[None, None, :]
                                 + bb[None, None, :])
                            mu = h.mean(axis=2)
                            var = h.var(axis=2)
                            mx = h.max(axis=2)
                            lse = mx + np.log(
                                np.exp(h - mx[:, :, None]).sum(axis=2))
                            if ((mu + var / 2.0 + math.log(k) - lse).max()
                                    > OCAP):
                                ok = False
                                break
                        if ok:
                            break
                    k = k - 1 if k <= 4 else int(k * 0.7)
                k = max(k, 1)
                groups.append((i, k))
                i += k
            mg = []
            for i0, k in groups:
                tt, gg, bb = txpk[i0:i0 + k], g1k[i0:i0 + k], Bpk[i0:i0 + k]
                mg.append((tt.mean(), gg.mean(),
                           bb.mean() + math.log(k) + bb.var() / 2.0,
                           tt.var() / 2.0, gg.var() / 2.0,
                           np.mean((tt - tt.mean()) * (gg - gg.mean())),
                           np.mean((tt - tt.mean()) * (bb - bb.mean())),
                           np.mean((gg - gg.mean()) * (bb - bb.mean())),
                           Gk[i0:i0 + k].min()))
            cols = [np.array(v) for v in zip(*mg)]
            keep2 = cols[8] * 4.0 * max(ylo, 1e-9) / sn2 < WSKIP
            tile_terms.append((cols, keep2))

        c1 = sum(len(tt[0][0]) for tt in tile_terms if tt)
        c2 = sum(int(tt[1].sum()) for tt in tile_terms if tt)
        tiles.append({"ylo": ylo, "yhi": yhi, "terms": tile_terms,
                      "c1": c1, "c2": c2})

    # process tiles in slot order = c1-ascending: batches of like-width
    # tiles (minimal padding), the narrow ones first so the pipeline
    # ramps while the DMA stream is still running and the wide batches
    # execute contention-free afterwards.  Host maps slots back later.
    slot_tiles = sorted(range(T), key=lambda t: tiles[t]["c1"])
    batches = []
    bstart = 0
    while bstart < T:
        bend = bstart + 1
        best = bstart + 1
        while bend <= T:
            sel = [tiles[slot_tiles[i]] for i in range(bstart, bend)]
            c1m = max(tl["c1"] for tl in sel)
            c2m = max(tl["c2"] for tl in sel)
            Bn = bend - bstart
            if Bn * (c1m + 3 + c2m) > PSUM_BUDGET or Bn > BMAX:
                break
            best = bend
            bend += 1
        bend = best
        sel = [tiles[slot_tiles[i]] for i in range(bstart, bend)]
        c1m = max(tl["c1"] for tl in sel)
        c2m = max(tl["c2"] for tl in sel)
        c1m += (c1m + 3) % 2
        batches.append((bstart, bend, c1m, c2m))
        bstart = bend

    return {"order_p": order_p, "mask_p": mask_p, "tiles": tiles,
            "batches": batches, "slot_tiles": slot_tiles,
            "logw": logw, "sn": sn, "c_u": c_u,
            "I": (I1, I2, I3)}


def _term_block(cols, sel, sgn):
    """rhs columns [ROWS, n] for one term's merged samples.  The ctb/cgb
    covariance corrections are folded into the t/g rows before the hi/lo
    split (they pair with the same x/y lhsT planes)."""
    tm, gm, Bm, vt, vg, ctg, ctb, cgb, _g = cols
    th, tl = _split(tm[sel] + ctb[sel])
    gh, gl = _split(sgn * (gm[sel] + cgb[sel]))
    bh, bl = _split(Bm[sel])
    n = len(th)
    one = np.ones(n, BF16)
    z = np.zeros(n, BF16)
    return np.stack([
        th, tl, th, gh, gl, gh, bh, bl,
        vt[sel].astype(BF16), vg[sel].astype(BF16),
        (sgn * ctg[sel]).astype(BF16),
        one, one, z, z,
    ]).astype(BF16)


def _pack(pl, x, y):
    """Build rhs [ROWS, NT] (shared), per-core lt [ROWS,T,P], mask, bvec."""
    sn = pl["sn"]; sn2 = sn * sn
    I1, I2, I3 = pl["I"]
    logw = pl["logw"]
    xs = np.asarray(x, np.float64)[pl["order_p"]]
    ysrt = np.asarray(y, np.float64)[pl["order_p"]]

    lny = np.log(ysrt)
    A = lny - 0.5 * (xs / sn) ** 2 - (ysrt / sn) ** 2
    Ps = [_interior_logp(xs, ysrt, I, sn) + logw[k]
          for k, I in enumerate((I1, I2, I3))]
    b_m = np.maximum.reduce(Ps + [lny + pl["c_u"]])
    nu64 = b_m - A
    nmh, nml = _split(-nu64)
    nu_use = -(nmh.astype(np.float64) + nml.astype(np.float64))
    bvec = (A + nu_use)
    lh, ll = _split(lny)

    ck = (math.log(2.0) - math.lgamma(1.5) - 4.0 * math.log(sn)
          - 0.5 * LOG2PI)
    intcols = np.zeros((ROWS, 3), BF16)
    for k, I in enumerate((I1, I2, I3)):
        tih, til = _split(np.array([I / sn2]))
        csth, cstl = _split(np.array([ck + logw[k] - 0.5 * I * I / sn2]))
        col = np.zeros(ROWS, BF16)
        col[0], col[1], col[2] = tih[0], til[0], tih[0]
        col[6], col[7] = csth[0], cstl[0]
        col[11] = col[12] = col[13] = col[14] = BF16(1.0)
        intcols[:, k] = col
    deadcol = np.zeros(ROWS, BF16)
    deadcol[6] = BF16(DEAD_B)

    # rhs: batch-major in SLOT order (slot s -> tile slot_tiles[s]),
    # per tile [C1' R1-cols | 3 interior | C2' R2-cols]
    slot_tiles = pl["slot_tiles"]
    rhs_parts = []
    meta = []
    off = 0
    for (t0, t1, C1, C2) in pl["batches"]:
        CW = C1 + 3 + C2
        for t in range(t0, t1):
            tt = pl["tiles"][slot_tiles[t]]
            blocks = []
            n1 = 0
            for j in range(3):
                ter = tt["terms"][j]
                if ter is None:
                    continue
                cols, keep2 = ter
                blk = _term_block(cols, slice(None), 1.0)
                blocks.append(blk)
                n1 += blk.shape[1]
            if n1 < C1:
                blocks.append(np.repeat(deadcol[:, None], C1 - n1, axis=1))
            blocks.append(intcols)
            n2 = 0
            for j in range(3):
                ter = tt["terms"][j]
                if ter is None:
                    continue
                cols, keep2 = ter
                if keep2.any():
                    blk = _term_block(cols, np.where(keep2)[0], -1.0)
                    blocks.append(blk)
                    n2 += blk.shape[1]
            if n2 < C2:
                blocks.append(np.repeat(deadcol[:, None], C2 - n2, axis=1))
            rhs_parts.append(np.concatenate(blocks, axis=1))
        meta.append((t0, t1, C1, C2, off))
        off += (t1 - t0) * CW
    rhs = np.concatenate(rhs_parts, axis=1).astype(BF16)
    NT = rhs.shape[1]
    assert NT == off

    xh, xl = _split(xs); yh, yl = _split(ysrt)
    planes = [
        xh, xh, xl, yh, yh, yl,
        np.ones(M_PAD, BF16), np.ones(M_PAD, BF16),
        (xs * xs).astype(BF16), (ysrt * ysrt).astype(BF16),
        (xs * ysrt).astype(BF16), nmh, nml, lh, ll,
    ]
    # pad slots: all-zero planes -> R = 0 for every column -> the slot
    # contributes exactly ln(C1'+3-C2') to the device sum (host-corrected)
    padm = pl["mask_p"] == 0.0
    planes = [np.where(padm, np.zeros(1, BF16), p).astype(BF16)
              for p in planes]

    # safety: packed R - nu must stay well under f32 exp overflow
    pf = np.stack([p.astype(np.float32) for p in planes]).astype(np.float64)
    rf = rhs.astype(np.float64)
    vmax = -1e30
    for (t0, t1, C1, C2, off0) in meta:
        CW = C1 + 3 + C2
        for s, t in enumerate(range(t0, t1)):
            tt = slot_tiles[t]
            blk = slice(tt * BLK, (tt + 1) * BLK)
            Rt = pf[:, blk].T @ rf[:, off0 + s * CW: off0 + (s + 1) * CW]
            vmax = max(vmax, float(Rt.max()))
    assert vmax < 60.0, f"exp overflow risk: max(R-nu)={vmax:.1f}"

    in_maps = []
    cores_bvec = []
    cores_mask = []
    st = np.asarray(slot_tiles)
    for i in range(N_CORES):
        lt = np.empty((ROWS, T, P), BF16)
        for r in range(ROWS):
            lt[r] = planes[r].reshape(T, N_CORES, P)[st, i, :]
        mask_i = np.ascontiguousarray(
            pl["mask_p"].reshape(T, N_CORES, P)[st, i, :].T).astype(
                np.float32)
        bvec_i = bvec.reshape(T, N_CORES, P)[st, i, :].T
        in_maps.append({"lt": lt, "rhs": rhs})
        cores_mask.append(mask_i)
        cores_bvec.append(bvec_i)
    return in_maps, meta, NT, cores_mask, cores_bvec


def _build(meta, NT):
    nc = bacc.Bacc("TRN2", target_bir_lowering=False, debug=False,
                   num_devices=N_CORES)
    dt_ = mybir.dt.float32
    bf = mybir.dt.bfloat16
    f = mybir.ActivationFunctionType
    alu = mybir.AluOpType

    lt_d = nc.dram_tensor("lt", [ROWS, T, P], bf, kind="ExternalInput").ap()
    rhs_d = nc.dram_tensor("rhs", [ROWS, NT], bf, kind="ExternalInput").ap()
    out_d = nc.dram_tensor("out", [P, T], dt_, kind="ExternalOutput").ap()

    def bank_slices(a, b):
        out = []
        while a < b:
            c = min(b, (a // 512 + 1) * 512)
            out.append((a, c))
            a = c
        return out

    with tile.TileContext(nc) as tc:
        with (
            tc.tile_pool(name="singles", bufs=1) as singles,
            tc.tile_pool(name="work", bufs=2) as work,
            tc.tile_pool(name="psum", bufs=2, space="PSUM") as psum_pool,
            tc.tile_pool(name="dump", bufs=2) as dump_pool,
            tc.tile_pool(name="fold", bufs=2) as fold_pool,
        ):
            lt = singles.tile([ROWS, T, P], bf, tag="lt")
            rhs = singles.tile([ROWS, NT], bf, tag="rhs")
            # per-batch chunks, round-robin across the three DMA-capable
            # engines; batch 0's pair goes on the two HW-DGE queues so
            # the pipeline starts as soon as its chunks land
            engs = [nc.sync, nc.scalar, nc.gpsimd]
            qi = 2
            nb = len(meta)
            edges = [meta[0][0]] + [m[1] for m in meta]
            for ci in range(nb):
                a, b = edges[ci], edges[ci + 1]
                (t0, t1, C1, C2, off0) = meta[ci]
                w = (t1 - t0) * (C1 + 3 + C2)
                if ci == 0:
                    nc.sync.dma_start(lt[:, a:b, :], lt_d[:, a:b, :])
                    nc.scalar.dma_start(rhs[:, off0:off0 + w],
                                        rhs_d[:, off0:off0 + w])
                    continue
                engs[qi % 3].dma_start(lt[:, a:b, :], lt_d[:, a:b, :])
                qi += 1
                engs[qi % 3].dma_start(rhs[:, off0:off0 + w],
                                       rhs_d[:, off0:off0 + w])
                qi += 1

            S1 = singles.tile([P, T], dt_, tag="S1")
            S2 = singles.tile([P, T], dt_, tag="S2")
            nc.vector.memset(S2[:], 0.0)
            zb = singles.tile([P, 1], dt_, tag="zb")
            nc.vector.memset(zb[:], 0.0)
            sd = singles.tile([P, T], dt_, tag="sd")

            for (t0, t1, C1, C2, off0) in meta:
                CW = C1 + 3 + C2
                Bn = t1 - t0
                Wb = Bn * CW
                ps = psum_pool.tile([P, Wb], dt_, tag="ps", name="ps")
                dp = dump_pool.tile([P, Wb], dt_, tag="dp", name="dp")
                for s in range(Bn):
                    for a, b in bank_slices(s * CW, (s + 1) * CW):
                        nc.tensor.matmul(ps[:, a:b], lt[:, t0 + s, :],
                                         rhs[:, off0 + a:off0 + b],
                                         start=True, stop=True)
                nc.scalar.activation(dp[:], ps[:], f.Exp, bias=zb[:])
                r3 = dp.rearrange("p (b c) -> p b c", c=CW)
                # pairwise fold on the (otherwise idle) gpsimd engine
                # halves the Vector reduce work
                h = (C1 + 3) // 2
                fl = fold_pool.tile([P, Bn * h], dt_, tag="fl", name="fl")
                f3 = fl.rearrange("p (b c) -> p b c", c=h)
                nc.gpsimd.tensor_tensor(f3[:], r3[:, :, 0:h],
                                        r3[:, :, h:2 * h], alu.add)
                nc.vector.tensor_reduce(S1[:, t0:t1], f3[:],
                                        mybir.AxisListType.X, alu.add)
                if C2 > 0:
                    nc.vector.tensor_reduce(S2[:, t0:t1],
                                            r3[:, :, C1 + 3:CW],
                                            mybir.AxisListType.X, alu.add)
                    nc.vector.scalar_tensor_tensor(
                        sd[:, t0:t1], S2[:, t0:t1], -1.0, S1[:, t0:t1],
                        alu.mult, alu.add)
                else:
                    nc.vector.tensor_copy(sd[:, t0:t1], S1[:, t0:t1])

            nc.sync.dma_start(out_d, sd[:])

    nc.compile()
    return nc


def kernel(x, y, ku12, ku23, ku13, sigma_b, sigma_n, I1, I2, I3, w):
    pl = _plan(x, y, ku12, ku23, ku13, sigma_b, sigma_n, I1, I2, I3, w)
    in_maps, meta, NT, cores_mask, cores_bvec = _pack(pl, x, y)

    key = (NT, tuple((m[0], m[1], m[2], m[3]) for m in meta))
    if key not in _graph_cache:
        _graph_cache[key] = _build(meta, NT)
    nc = _graph_cache[key]

    res = run_bass_kernel_spmd(nc, in_maps, core_ids=list(range(N_CORES)))
    global _last_results
    _last_results = res

    loss = 0.0
    for i in range(N_CORES):
        sd = np.asarray(res.results[i]["out"], np.float64)
        lm = np.log(np.maximum(sd, 1e-300)) + cores_bvec[i]
        loss += float((lm * cores_mask[i]).sum())
    return np.float32(-loss)


# revision 15
# speedup vs baseline: 1.0219x; 1.0219x over previous
"""Trainium2 Bass kernel for the ArcModel3Phase loss (y-sorted redesign).

Math: per point m, logmix = ln(sum_j e^{l_j}) over 6 mixture components
(3 interior Gaussians + 3 MC-integrated interface terms of N=1024 samples
each).  Writing l = A(x,y) + h with A = lny - x^2/2sn^2 - y^2/sn^2 and h
affine in (x, y, lny, 1), every component (and the per-m bias) becomes a
column of ONE bf16 matmul over 17 lhsT rows:

  R[p, c] = sum_k lhsT[k, p] rhs[k, c]   -> exp -> segmented row sums.

Device work per point is ~100 columns instead of 3072 thanks to:
  1. Global y-sort (host permutes; the loss is a sum over m, so no
     unpermute).  Each 1024-point block has a narrow y-range, so most MC
     samples are irrelevant to it: a sample contributes only within
     |y - G(tx)| ~ 0.2.  Host prunes per block against a logmix lower
     bound on an x-grid (cutoff e^-PRUNE).
  2. Adaptive sample merging (2nd-order cumulant, exact variance carried
     as 5 extra matmul rows) with a per-block relevance window, plus an
     overshoot guard that keeps each merged column within OCAP of the
     exact logsumexp at window probes (prevents f32 exp overflow and
     bounds the merge error).
  3. The e^{R2} subtraction pass (Bessel 1-e^{-w} expansion) is skipped
     for samples with w = 4yG/sn^2 >= WSKIP for the whole block - almost
     all of them once y is sorted.
  4. The per-m exp bias nu = b - A (b = max of per-component upper
     bounds, a tight cover of max_j l_j) is pure host math, folded into
     the matmul as two hi/lo bf16 rows.  No on-device max pass at all.
  5. Interior components are affine in (x, lny): 3 more columns, two
     lny rows.  The final ln + masked sum runs on host from the DMA'd
     [128, T] mix tile (f64, more accurate than device f32 accum).

One EXP instruction covers a whole batch of tiles (PSUM budget 2048
f32), then two segmented DVE reduces produce S1 (R1+interior) and S2
per tile; mix = S1 - S2.
"""
import math

import numpy as np
import ml_dtypes
from scipy.special import erf, erfinv

import concourse.bass as bass
import concourse.tile as tile
from concourse import bacc, mybir
from concourse.bass_utils import run_bass_kernel_spmd

BF16 = ml_dtypes.bfloat16
WF = 3.0
LOG2PI = math.log(2.0 * math.pi)
M = 100_000
N_MC = 1024
P = 128
N_CORES = 8
BLK = P * N_CORES              # 1024 points per global block
T = (M + BLK - 1) // BLK       # 98 tiles per core
M_PAD = T * BLK
ROWS = 15
DEAD_B = -30000.0

DM = 24.0                      # max in-window |h - mean| within a group
KMAX = 96                      # max group size
PRUNE = 7.0                    # per-block relevance cutoff (e-folds)
WSKIP = 9.0                    # skip R2 columns with w >= this block-wide
OCAP = 3.5                     # max merged-vs-exact LSE overshoot
XWIN = 0.40                    # merge relevance half-window in x
PSUM_BUDGET = 2048             # f32 columns per batch (4 PSUM banks)
BMAX = 16                      # max tiles per batch

_graph_cache = {}
_last_results = None


def _split(a):
    hi = np.asarray(a).astype(BF16)
    lo = (np.asarray(a, np.float64) - hi.astype(np.float64)).astype(BF16)
    return hi, lo


def _host_rows(ku, Ia, Ib, sigma_b, sigma_n, logw):
    """Raw per-sample rows for one interface term (float64, tx-sorted)."""
    ku = np.asarray(ku, np.float64)
    sn2 = sigma_n ** 2
    I_min = Ia + 0.5 * (Ib - Ia) * (1.0 + erf(-WF / np.sqrt(2.0)))
    I_diff = (Ib - Ia) * erf(WF / np.sqrt(2.0))
    tx = np.sort(ku * I_diff + I_min)
    ei = erfinv(2.0 * (tx - Ia) / (Ib - Ia) - 1.0)
    G = (Ib - Ia) / np.sqrt(2.0 * np.pi * sigma_b ** 2) * np.exp(-ei ** 2)
    lptx = -np.log(2.0 * WF * (Ib - Ia)) + 0.5 * LOG2PI + ei ** 2
    B = -0.5 * tx ** 2 / sn2 - np.log(G) - G ** 2 / sn2 + lptx
    C0 = (-np.log(sigma_n) - 0.5 * LOG2PI
          + np.log(2.0) - 2.0 * np.log(sigma_n)
          + 0.5 * np.log(2.0 / np.pi) - np.log(2.0)
          - 0.5 * np.log(2.0) + np.log(sigma_n))
    Bp = B + np.log(I_diff) - np.log(N_MC) + logw + C0
    return tx, tx / sn2, 2.0 * G / sn2, Bp, G


def _raw_l(xg, yv, term, sn2):
    """l_n(xg, yv) for all samples of one term: [X, N]."""
    tx, txp, g1, Bp, G = term
    w = np.minimum(4.0 * yv * G / sn2, 700.0)
    return (Bp[None, :] + xg[:, None] * txp[None, :] + yv * g1[None, :]
            + np.log1p(-np.exp(-w))[None, :]
            + np.log(yv) - 0.5 * (xg[:, None] ** 2) / sn2 - yv * yv / sn2)


def _interior_logp(x, y, I, sn):
    return (math.log(2.0) + 2.0 * np.log(y) - math.lgamma(1.5)
            - 3.0 * math.log(sn) - (y / sn) ** 2
            - math.log(sn) - 0.5 * LOG2PI - 0.5 * ((x - I) / sn) ** 2)


def _plan(x, y, ku12, ku23, ku13, sigma_b, sigma_n, I1, I2, I3, w):
    x = np.asarray(x, np.float64)
    y = np.asarray(y, np.float64)
    sn = float(sigma_n); sb = float(sigma_b)
    I1, I2, I3 = float(I1), float(I2), float(I3)
    w64 = np.asarray(w, np.float64)
    logw = w64 - (np.log(np.sum(np.exp(w64 - w64.max()))) + w64.max())
    sn2 = sn * sn

    terms = [_host_rows(ku, Ia, Ib, sb, sn, float(logw[3 + j]))
             for j, (ku, Ia, Ib) in enumerate(
                 ((ku12, I1, I2), (ku23, I2, I3), (ku13, I1, I3)))]

    # l(x,y) <= lny + c_u: per-sample peak at (tx, G), minus its lny part
    c_u = -1e30
    for tx, txp, g1, Bp, G in terms:
        l_peak = (np.log(G) + 0.5 * tx ** 2 / sn2 + G ** 2 / sn2 + Bp
                  + np.log1p(-np.exp(-np.minimum(4.0 * G * G / sn2, 700.0))))
        c_u = max(c_u, float((l_peak - np.log(G)).max()))

    order = np.argsort(y, kind="stable")
    pad = M_PAD - len(x)
    order_p = np.concatenate([order, np.repeat(order[-1], pad)])
    mask_p = np.concatenate([np.ones(len(x), np.float32),
                             np.zeros(pad, np.float32)])
    ys = y[order_p]

    xmin, xmax = float(x.min()), float(x.max())
    xg = np.linspace(xmin, xmax, 121)

    def logmix_lb(yv):
        mx = np.maximum.reduce([_interior_logp(xg, yv, I, sn) + logw[k]
                                for k, I in enumerate((I1, I2, I3))])
        for term in terms:
            l = _raw_l(xg, yv, term, sn2)
            m2 = l.max(axis=1)
            mx = np.maximum(mx, m2 + np.log(
                np.sum(np.exp(l - m2[:, None]), axis=1)))
        return mx

    tiles = []
    for t in range(T):
        blk = slice(t * BLK, (t + 1) * BLK)
        yb = ys[blk]
        ylo, yhi = float(yb.min()), float(yb.max())
        yprobes = np.linspace(ylo, yhi, 3)
        lmix = np.max([logmix_lb(yv) for yv in yprobes], axis=0)

        tile_terms = []
        for term in terms:
            tx, txp, g1, Bp, G = term
            keep = np.zeros(len(tx), bool)
            for yv in yprobes:
                l = _raw_l(xg, yv, term, sn2)
                keep |= (l - lmix[:, None] >= -PRUNE).any(axis=0)
            yc = np.clip(G, ylo, yhi)
            wv = np.minimum(4.0 * yc * G / sn2, 700.0)
            l = (Bp[None, :] + xg[:, None] * txp[None, :]
                 + (yc * g1)[None, :] + np.log1p(-np.exp(-wv))[None, :]
                 + np.log(yc)[None, :] - 0.5 * (xg[:, None] ** 2) / sn2
                 - (yc * yc)[None, :] / sn2)
            keep |= (l - lmix[:, None] >= -PRUNE).any(axis=0)

            idx = np.where(keep)[0]
            if len(idx) == 0:
                tile_terms.append(None)
                continue
            txk, txpk, g1k, Bpk, Gk = (tx[idx], txp[idx], g1[idx],
                                       Bp[idx], G[idx])
            groups = []
            i = 0
            n = len(idx)
            while i < n:
                k = min(KMAX, n - i)
                while k > 1:
                    tt, gg, bb = txpk[i:i + k], g1k[i:i + k], Bpk[i:i + k]
                    txm = txk[i:i + k].mean()
                    lo = max(xmin, txm - XWIN); hi = min(xmax, txm + XWIN)
                    dt = tt - tt.mean(); db = bb - bb.mean()
                    dg = np.abs(gg - gg.mean())
                    d = (np.maximum(np.abs(dt * lo + db),
                                    np.abs(dt * hi + db)) + dg * yhi)
                    if d.max() <= DM:
                        xp = np.array([lo, 0.5 * (lo + hi), hi])
                        yp = np.array([ylo, yhi])
                        ok = True
                        for sgn in (1.0, -1.0):
                            h = (xp[:, None, None] * tt[None, None, :]
                                 + sgn * yp[None, :, None] * gg[None, None, :]
                                 + bb[None, None, :])
                            mu = h.mean(axis=2)
                            var = h.var(axis=2)
                            mx = h.max(axis=2)
                            lse = mx + np.log(
                                np.exp(h - mx[:, :, None]).sum(axis=2))
                            if ((mu + var / 2.0 + math.log(k) - lse).max()
                                    > OCAP):
                                ok = False
                                break
                        if ok:
                            break
                    k = k - 1 if k <= 4 else int(k * 0.7)
                k = max(k, 1)
                groups.append((i, k))
                i += k
            mg = []
            for i0, k in groups:
                tt, gg, bb = txpk[i0:i0 + k], g1k[i0:i0 + k], Bpk[i0:i0 + k]
                mg.append((tt.mean(), gg.mean(),
                           bb.mean() + math.log(k) + bb.var() / 2.0,
                           tt.var() / 2.0, gg.var() / 2.0,
                           np.mean((tt - tt.mean()) * (gg - gg.mean())),
                           np.mean((tt - tt.mean()) * (bb - bb.mean())),
                           np.mean((gg - gg.mean()) * (bb - bb.mean())),
                           Gk[i0:i0 + k].min()))
            cols = [np.array(v) for v in zip(*mg)]
            keep2 = cols[8] * 4.0 * max(ylo, 1e-9) / sn2 < WSKIP
            tile_terms.append((cols, keep2))

        c1 = sum(len(tt[0][0]) for tt in tile_terms if tt)
        c2 = sum(int(tt[1].sum()) for tt in tile_terms if tt)
        tiles.append({"ylo": ylo, "yhi": yhi, "terms": tile_terms,
                      "c1": c1, "c2": c2})

    # process tiles in slot order = c1-ascending: batches of like-width
    # tiles (minimal padding), the narrow ones first so the pipeline
    # ramps while the DMA stream is still running and the wide batches
    # execute contention-free afterwards.  Host maps slots back later.
    slot_tiles = sorted(range(T), key=lambda t: tiles[t]["c1"])
    batches = []
    bstart = 0
    while bstart < T:
        bend = bstart + 1
        best = bstart + 1
        while bend <= T:
            sel = [tiles[slot_tiles[i]] for i in range(bstart, bend)]
            c1m = max(tl["c1"] for tl in sel)
            c2m = max(tl["c2"] for tl in sel)
            Bn = bend - bstart
            if Bn * (c1m + 3 + c2m) > PSUM_BUDGET or Bn > BMAX:
                break
            best = bend
            bend += 1
        bend = best
        sel = [tiles[slot_tiles[i]] for i in range(bstart, bend)]
        c1m = max(tl["c1"] for tl in sel)
        c2m = max(tl["c2"] for tl in sel)
        c1m += (c1m + 3) % 2
        batches.append((bstart, bend, c1m, c2m))
        bstart = bend

    return {"order_p": order_p, "mask_p": mask_p, "tiles": tiles,
            "batches": batches, "slot_tiles": slot_tiles,
            "logw": logw, "sn": sn, "c_u": c_u,
            "I": (I1, I2, I3)}


def _term_block(cols, sel, sgn):
    """rhs columns [ROWS, n] for one term's merged samples.  The ctb/cgb
    covariance corrections are folded into the t/g rows before the hi/lo
    split (they pair with the same x/y lhsT planes)."""
    tm, gm, Bm, vt, vg, ctg, ctb, cgb, _g = cols
    th, tl = _split(tm[sel] + ctb[sel])
    gh, gl = _split(sgn * (gm[sel] + cgb[sel]))
    bh, bl = _split(Bm[sel])
    n = len(th)
    one = np.ones(n, BF16)
    z = np.zeros(n, BF16)
    return np.stack([
        th, tl, th, gh, gl, gh, bh, bl,
        vt[sel].astype(BF16), vg[sel].astype(BF16),
        (sgn * ctg[sel]).astype(BF16),
        one, one, z, z,
    ]).astype(BF16)


def _pack(pl, x, y):
    """Build rhs [ROWS, NT] (shared), per-core lt [ROWS,T,P], mask, bvec."""
    sn = pl["sn"]; sn2 = sn * sn
    I1, I2, I3 = pl["I"]
    logw = pl["logw"]
    xs = np.asarray(x, np.float64)[pl["order_p"]]
    ysrt = np.asarray(y, np.float64)[pl["order_p"]]

    lny = np.log(ysrt)
    A = lny - 0.5 * (xs / sn) ** 2 - (ysrt / sn) ** 2
    Ps = [_interior_logp(xs, ysrt, I, sn) + logw[k]
          for k, I in enumerate((I1, I2, I3))]
    b_m = np.maximum.reduce(Ps + [lny + pl["c_u"]])
    nu64 = b_m - A
    nmh, nml = _split(-nu64)
    nu_use = -(nmh.astype(np.float64) + nml.astype(np.float64))
    bvec = (A + nu_use)
    lh, ll = _split(lny)

    ck = (math.log(2.0) - math.lgamma(1.5) - 4.0 * math.log(sn)
          - 0.5 * LOG2PI)
    intcols = np.zeros((ROWS, 3), BF16)
    for k, I in enumerate((I1, I2, I3)):
        tih, til = _split(np.array([I / sn2]))
        csth, cstl = _split(np.array([ck + logw[k] - 0.5 * I * I / sn2]))
        col = np.zeros(ROWS, BF16)
        col[0], col[1], col[2] = tih[0], til[0], tih[0]
        col[6], col[7] = csth[0], cstl[0]
        col[11] = col[12] = col[13] = col[14] = BF16(1.0)
        intcols[:, k] = col
    deadcol = np.zeros(ROWS, BF16)
    deadcol[6] = BF16(DEAD_B)

    # rhs: batch-major in SLOT order (slot s -> tile slot_tiles[s]),
    # per tile [C1' R1-cols | 3 interior | C2' R2-cols]
    slot_tiles = pl["slot_tiles"]
    rhs_parts = []
    meta = []
    off = 0
    for (t0, t1, C1, C2) in pl["batches"]:
        CW = C1 + 3 + C2
        for t in range(t0, t1):
            tt = pl["tiles"][slot_tiles[t]]
            blocks = []
            n1 = 0
            for j in range(3):
                ter = tt["terms"][j]
                if ter is None:
                    continue
                cols, keep2 = ter
                blk = _term_block(cols, slice(None), 1.0)
                blocks.append(blk)
                n1 += blk.shape[1]
            if n1 < C1:
                blocks.append(np.repeat(deadcol[:, None], C1 - n1, axis=1))
            blocks.append(intcols)
            n2 = 0
            for j in range(3):
                ter = tt["terms"][j]
                if ter is None:
                    continue
                cols, keep2 = ter
                if keep2.any():
                    blk = _term_block(cols, np.where(keep2)[0], -1.0)
                    blocks.append(blk)
                    n2 += blk.shape[1]
            if n2 < C2:
                blocks.append(np.repeat(deadcol[:, None], C2 - n2, axis=1))
            rhs_parts.append(np.concatenate(blocks, axis=1))
        meta.append((t0, t1, C1, C2, off))
        off += (t1 - t0) * CW
    rhs = np.concatenate(rhs_parts, axis=1).astype(BF16)
    NT = rhs.shape[1]
    assert NT == off

    xh, xl = _split(xs); yh, yl = _split(ysrt)
    planes = [
        xh, xh, xl, yh, yh, yl,
        np.ones(M_PAD, BF16), np.ones(M_PAD, BF16),
        (xs * xs).astype(BF16), (ysrt * ysrt).astype(BF16),
        (xs * ysrt).astype(BF16), nmh, nml, lh, ll,
    ]
    # pad slots: all-zero planes -> R = 0 for every column -> the slot
    # contributes exactly ln(C1'+3-C2') to the device sum (host-corrected)
    padm = pl["mask_p"] == 0.0
    planes = [np.where(padm, np.zeros(1, BF16), p).astype(BF16)
              for p in planes]

    # safety: packed R - nu must stay well under f32 exp overflow
    pf = np.stack([p.astype(np.float32) for p in planes]).astype(np.float64)
    rf = rhs.astype(np.float64)
    vmax = -1e30
    for (t0, t1, C1, C2, off0) in meta:
        CW = C1 + 3 + C2
        for s, t in enumerate(range(t0, t1)):
            tt = slot_tiles[t]
            blk = slice(tt * BLK, (tt + 1) * BLK)
            Rt = pf[:, blk].T @ rf[:, off0 + s * CW: off0 + (s + 1) * CW]
            vmax = max(vmax, float(Rt.max()))
    assert vmax < 60.0, f"exp overflow risk: max(R-nu)={vmax:.1f}"

    in_maps = []
    cores_bvec = []
    cores_mask = []
    st = np.asarray(slot_tiles)
    for i in range(N_CORES):
        lt = np.empty((ROWS, T, P), BF16)
        for r in range(ROWS):
            lt[r] = planes[r].reshape(T, N_CORES, P)[st, i, :]
        mask_i = np.ascontiguousarray(
            pl["mask_p"].reshape(T, N_CORES, P)[st, i, :].T).astype(
                np.float32)
        bvec_i = bvec.reshape(T, N_CORES, P)[st, i, :].T
        in_maps.append({"lt": lt, "rhs": rhs})
        cores_mask.append(mask_i)
        cores_bvec.append(bvec_i)
    return in_maps, meta, NT, cores_mask, cores_bvec


def _build(meta, NT):
    nc = bacc.Bacc("TRN2", target_bir_lowering=False, debug=False,
                   num_devices=N_CORES)
    dt_ = mybir.dt.float32
    bf = mybir.dt.bfloat16
    f = mybir.ActivationFunctionType
    alu = mybir.AluOpType

    lt_d = nc.dram_tensor("lt", [ROWS, T, P], bf, kind="ExternalInput").ap()
    rhs_d = nc.dram_tensor("rhs", [ROWS, NT], bf, kind="ExternalInput").ap()
    out_d = nc.dram_tensor("out", [P, T], dt_, kind="ExternalOutput").ap()

    def bank_slices(a, b):
        out = []
        while a < b:
            c = min(b, (a // 512 + 1) * 512)
            out.append((a, c))
            a = c
        return out

    with tile.TileContext(nc) as tc:
        with (
            tc.tile_pool(name="singles", bufs=1) as singles,
            tc.tile_pool(name="work", bufs=2) as work,
            tc.tile_pool(name="psum", bufs=2, space="PSUM") as psum_pool,
            tc.tile_pool(name="dump", bufs=2) as dump_pool,
            tc.tile_pool(name="fold", bufs=2) as fold_pool,
        ):
            lt = singles.tile([ROWS, T, P], bf, tag="lt")
            rhs = singles.tile([ROWS, NT], bf, tag="rhs")
            # per-batch chunks, round-robin across the three DMA-capable
            # engines; batch 0's pair goes on the two HW-DGE queues so
            # the pipeline starts as soon as its chunks land
            engs = [nc.sync, nc.scalar, nc.gpsimd]
            qi = 2
            nb = len(meta)
            edges = [meta[0][0]] + [m[1] for m in meta]
            for ci in range(nb):
                a, b = edges[ci], edges[ci + 1]
                (t0, t1, C1, C2, off0) = meta[ci]
                w = (t1 - t0) * (C1 + 3 + C2)
                if ci == 0:
                    nc.sync.dma_start(lt[:, a:b, :], lt_d[:, a:b, :])
                    nc.scalar.dma_start(rhs[:, off0:off0 + w],
                                        rhs_d[:, off0:off0 + w])
                    continue
                engs[qi % 3].dma_start(lt[:, a:b, :], lt_d[:, a:b, :])
                qi += 1
                engs[qi % 3].dma_start(rhs[:, off0:off0 + w],
                                       rhs_d[:, off0:off0 + w])
                qi += 1

            S1 = singles.tile([P, T], dt_, tag="S1")
            S2 = singles.tile([P, T], dt_, tag="S2")
            nc.vector.memset(S2[:], 0.0)
            zb = singles.tile([P, 1], dt_, tag="zb")
            nc.vector.memset(zb[:], 0.0)
            sd = singles.tile([P, T], dt_, tag="sd")

            for (t0, t1, C1, C2, off0) in meta:
                CW = C1 + 3 + C2
                Bn = t1 - t0
                Wb = Bn * CW
                ps = psum_pool.tile([P, Wb], dt_, tag="ps", name="ps")
                dp = dump_pool.tile([P, Wb], dt_, tag="dp", name="dp")
                for s in range(Bn):
                    for a, b in bank_slices(s * CW, (s + 1) * CW):
                        nc.tensor.matmul(ps[:, a:b], lt[:, t0 + s, :],
                                         rhs[:, off0 + a:off0 + b],
                                         start=True, stop=True)
                nc.scalar.activation(dp[:], ps[:], f.Exp, bias=zb[:])
                r3 = dp.rearrange("p (b c) -> p b c", c=CW)
                # pairwise fold on the (otherwise idle) gpsimd engine
                # halves the Vector reduce work
                h = (C1 + 3) // 2
                fl = fold_pool.tile([P, Bn * h], dt_, tag="fl", name="fl")
                f3 = fl.rearrange("p (b c) -> p b c", c=h)
                nc.vector.tensor_tensor(f3[:], r3[:, :, 0:h],
                                        r3[:, :, h:2 * h], alu.add)
                nc.vector.tensor_reduce(S1[:, t0:t1], f3[:],
                                        mybir.AxisListType.X, alu.add)
                if C2 > 0:
                    nc.vector.tensor_reduce(S2[:, t0:t1],
                                            r3[:, :, C1 + 3:CW],
                                            mybir.AxisListType.X, alu.add)

            nc.vector.scalar_tensor_tensor(sd[:], S2[:], -1.0, S1[:],
                                           alu.mult, alu.add)
            nc.sync.dma_start(out_d, sd[:])

    nc.compile()
    return nc


def kernel(x, y, ku12, ku23, ku13, sigma_b, sigma_n, I1, I2, I3, w):
    pl = _plan(x, y, ku12, ku23, ku13, sigma_b, sigma_n, I1, I2, I3, w)
    in_maps, meta, NT, cores_mask, cores_bvec = _pack(pl, x, y)

    key = (NT, tuple((m[0], m[1], m[2], m[3]) for m in meta))
    if key not in _graph_cache:
        _graph_cache[key] = _build(meta, NT)
    nc = _graph_cache[key]

    res = run_bass_kernel_spmd(nc, in_maps, core_ids=list(range(N_CORES)))
    global _last_results
    _last_results = res

    loss = 0.0
    for i in range(N_CORES):
        sd = np.asarray(res.results[i]["out"], np.float64)
        lm = np.log(np.maximum(sd, 1e-300)) + cores_bvec[i]
        loss += float((lm * cores_mask[i]).sum())
    return np.float32(-loss)


# revision 16
# speedup vs baseline: 1.0469x; 1.0245x over previous
"""Trainium2 Bass kernel for the ArcModel3Phase loss (y-sorted redesign).

Math: per point m, logmix = ln(sum_j e^{l_j}) over 6 mixture components
(3 interior Gaussians + 3 MC-integrated interface terms of N=1024 samples
each).  Writing l = A(x,y) + h with A = lny - x^2/2sn^2 - y^2/sn^2 and h
affine in (x, y, lny, 1), every component (and the per-m bias) becomes a
column of ONE bf16 matmul over 17 lhsT rows:

  R[p, c] = sum_k lhsT[k, p] rhs[k, c]   -> exp -> segmented row sums.

Device work per point is ~100 columns instead of 3072 thanks to:
  1. Global y-sort (host permutes; the loss is a sum over m, so no
     unpermute).  Each 1024-point block has a narrow y-range, so most MC
     samples are irrelevant to it: a sample contributes only within
     |y - G(tx)| ~ 0.2.  Host prunes per block against a logmix lower
     bound on an x-grid (cutoff e^-PRUNE).
  2. Adaptive sample merging (2nd-order cumulant, exact variance carried
     as 5 extra matmul rows) with a per-block relevance window, plus an
     overshoot guard that keeps each merged column within OCAP of the
     exact logsumexp at window probes (prevents f32 exp overflow and
     bounds the merge error).
  3. The e^{R2} subtraction pass (Bessel 1-e^{-w} expansion) is skipped
     for samples with w = 4yG/sn^2 >= WSKIP for the whole block - almost
     all of them once y is sorted.
  4. The per-m exp bias nu = b - A (b = max of per-component upper
     bounds, a tight cover of max_j l_j) is pure host math, folded into
     the matmul as two hi/lo bf16 rows.  No on-device max pass at all.
  5. Interior components are affine in (x, lny): 3 more columns, two
     lny rows.  The final ln + masked sum runs on host from the DMA'd
     [128, T] mix tile (f64, more accurate than device f32 accum).

One EXP instruction covers a whole batch of tiles (PSUM budget 2048
f32), then two segmented DVE reduces produce S1 (R1+interior) and S2
per tile; mix = S1 - S2.
"""
import math

import numpy as np
import ml_dtypes
from scipy.special import erf, erfinv

import concourse.bass as bass
import concourse.tile as tile
from concourse import bacc, mybir
from concourse.bass_utils import run_bass_kernel_spmd

BF16 = ml_dtypes.bfloat16
WF = 3.0
LOG2PI = math.log(2.0 * math.pi)
M = 100_000
N_MC = 1024
P = 128
N_CORES = 8
BLK = P * N_CORES              # 1024 points per global block
T = (M + BLK - 1) // BLK       # 98 tiles per core
M_PAD = T * BLK
ROWS = 15
DEAD_B = -30000.0

DM = 24.0                      # max in-window |h - mean| within a group
KMAX = 96                      # max group size
PRUNE = 6.5                    # per-block relevance cutoff (e-folds)
WSKIP = 9.0                    # skip R2 columns with w >= this block-wide
OCAP = 4.0                     # max merged-vs-exact LSE overshoot
XWIN = 0.40                    # merge relevance half-window in x
PSUM_BUDGET = 2048             # f32 columns per batch (4 PSUM banks)
BMAX = 16                      # max tiles per batch

_graph_cache = {}
_last_results = None


def _split(a):
    hi = np.asarray(a).astype(BF16)
    lo = (np.asarray(a, np.float64) - hi.astype(np.float64)).astype(BF16)
    return hi, lo


def _host_rows(ku, Ia, Ib, sigma_b, sigma_n, logw):
    """Raw per-sample rows for one interface term (float64, tx-sorted)."""
    ku = np.asarray(ku, np.float64)
    sn2 = sigma_n ** 2
    I_min = Ia + 0.5 * (Ib - Ia) * (1.0 + erf(-WF / np.sqrt(2.0)))
    I_diff = (Ib - Ia) * erf(WF / np.sqrt(2.0))
    tx = np.sort(ku * I_diff + I_min)
    ei = erfinv(2.0 * (tx - Ia) / (Ib - Ia) - 1.0)
    G = (Ib - Ia) / np.sqrt(2.0 * np.pi * sigma_b ** 2) * np.exp(-ei ** 2)
    lptx = -np.log(2.0 * WF * (Ib - Ia)) + 0.5 * LOG2PI + ei ** 2
    B = -0.5 * tx ** 2 / sn2 - np.log(G) - G ** 2 / sn2 + lptx
    C0 = (-np.log(sigma_n) - 0.5 * LOG2PI
          + np.log(2.0) - 2.0 * np.log(sigma_n)
          + 0.5 * np.log(2.0 / np.pi) - np.log(2.0)
          - 0.5 * np.log(2.0) + np.log(sigma_n))
    Bp = B + np.log(I_diff) - np.log(N_MC) + logw + C0
    return tx, tx / sn2, 2.0 * G / sn2, Bp, G


def _raw_l(xg, yv, term, sn2):
    """l_n(xg, yv) for all samples of one term: [X, N]."""
    tx, txp, g1, Bp, G = term
    w = np.minimum(4.0 * yv * G / sn2, 700.0)
    return (Bp[None, :] + xg[:, None] * txp[None, :] + yv * g1[None, :]
            + np.log1p(-np.exp(-w))[None, :]
            + np.log(yv) - 0.5 * (xg[:, None] ** 2) / sn2 - yv * yv / sn2)


def _interior_logp(x, y, I, sn):
    return (math.log(2.0) + 2.0 * np.log(y) - math.lgamma(1.5)
            - 3.0 * math.log(sn) - (y / sn) ** 2
            - math.log(sn) - 0.5 * LOG2PI - 0.5 * ((x - I) / sn) ** 2)


def _plan(x, y, ku12, ku23, ku13, sigma_b, sigma_n, I1, I2, I3, w):
    x = np.asarray(x, np.float64)
    y = np.asarray(y, np.float64)
    sn = float(sigma_n); sb = float(sigma_b)
    I1, I2, I3 = float(I1), float(I2), float(I3)
    w64 = np.asarray(w, np.float64)
    logw = w64 - (np.log(np.sum(np.exp(w64 - w64.max()))) + w64.max())
    sn2 = sn * sn

    terms = [_host_rows(ku, Ia, Ib, sb, sn, float(logw[3 + j]))
             for j, (ku, Ia, Ib) in enumerate(
                 ((ku12, I1, I2), (ku23, I2, I3), (ku13, I1, I3)))]

    # l(x,y) <= lny + c_u: per-sample peak at (tx, G), minus its lny part
    c_u = -1e30
    for tx, txp, g1, Bp, G in terms:
        l_peak = (np.log(G) + 0.5 * tx ** 2 / sn2 + G ** 2 / sn2 + Bp
                  + np.log1p(-np.exp(-np.minimum(4.0 * G * G / sn2, 700.0))))
        c_u = max(c_u, float((l_peak - np.log(G)).max()))

    order = np.argsort(y, kind="stable")
    pad = M_PAD - len(x)
    order_p = np.concatenate([order, np.repeat(order[-1], pad)])
    mask_p = np.concatenate([np.ones(len(x), np.float32),
                             np.zeros(pad, np.float32)])
    ys = y[order_p]

    xmin, xmax = float(x.min()), float(x.max())
    xg = np.linspace(xmin, xmax, 121)

    def logmix_lb(yv):
        mx = np.maximum.reduce([_interior_logp(xg, yv, I, sn) + logw[k]
                                for k, I in enumerate((I1, I2, I3))])
        for term in terms:
            l = _raw_l(xg, yv, term, sn2)
            m2 = l.max(axis=1)
            mx = np.maximum(mx, m2 + np.log(
                np.sum(np.exp(l - m2[:, None]), axis=1)))
        return mx

    tiles = []
    for t in range(T):
        blk = slice(t * BLK, (t + 1) * BLK)
        yb = ys[blk]
        ylo, yhi = float(yb.min()), float(yb.max())
        yprobes = np.linspace(ylo, yhi, 3)
        lmix = np.max([logmix_lb(yv) for yv in yprobes], axis=0)

        tile_terms = []
        for term in terms:
            tx, txp, g1, Bp, G = term
            keep = np.zeros(len(tx), bool)
            for yv in yprobes:
                l = _raw_l(xg, yv, term, sn2)
                keep |= (l - lmix[:, None] >= -PRUNE).any(axis=0)
            yc = np.clip(G, ylo, yhi)
            wv = np.minimum(4.0 * yc * G / sn2, 700.0)
            l = (Bp[None, :] + xg[:, None] * txp[None, :]
                 + (yc * g1)[None, :] + np.log1p(-np.exp(-wv))[None, :]
                 + np.log(yc)[None, :] - 0.5 * (xg[:, None] ** 2) / sn2
                 - (yc * yc)[None, :] / sn2)
            keep |= (l - lmix[:, None] >= -PRUNE).any(axis=0)

            idx = np.where(keep)[0]
            if len(idx) == 0:
                tile_terms.append(None)
                continue
            txk, txpk, g1k, Bpk, Gk = (tx[idx], txp[idx], g1[idx],
                                       Bp[idx], G[idx])
            groups = []
            i = 0
            n = len(idx)
            while i < n:
                k = min(KMAX, n - i)
                while k > 1:
                    tt, gg, bb = txpk[i:i + k], g1k[i:i + k], Bpk[i:i + k]
                    txm = txk[i:i + k].mean()
                    lo = max(xmin, txm - XWIN); hi = min(xmax, txm + XWIN)
                    dt = tt - tt.mean(); db = bb - bb.mean()
                    dg = np.abs(gg - gg.mean())
                    d = (np.maximum(np.abs(dt * lo + db),
                                    np.abs(dt * hi + db)) + dg * yhi)
                    if d.max() <= DM:
                        xp = np.array([lo, 0.5 * (lo + hi), hi])
                        yp = np.array([ylo, yhi])
                        ok = True
                        for sgn in (1.0, -1.0):
                            h = (xp[:, None, None] * tt[None, None, :]
                                 + sgn * yp[None, :, None] * gg[None, None, :]
                                 + bb[None, None, :])
                            mu = h.mean(axis=2)
                            var = h.var(axis=2)
                            mx = h.max(axis=2)
                            lse = mx + np.log(
                                np.exp(h - mx[:, :, None]).sum(axis=2))
                            if ((mu + var / 2.0 + math.log(k) - lse).max()
                                    > OCAP):
                                ok = False
                                break
                        if ok:
                            break
                    k = k - 1 if k <= 4 else int(k * 0.7)
                k = max(k, 1)
                groups.append((i, k))
                i += k
            mg = []
            for i0, k in groups:
                tt, gg, bb = txpk[i0:i0 + k], g1k[i0:i0 + k], Bpk[i0:i0 + k]
                mg.append((tt.mean(), gg.mean(),
                           bb.mean() + math.log(k) + bb.var() / 2.0,
                           tt.var() / 2.0, gg.var() / 2.0,
                           np.mean((tt - tt.mean()) * (gg - gg.mean())),
                           np.mean((tt - tt.mean()) * (bb - bb.mean())),
                           np.mean((gg - gg.mean()) * (bb - bb.mean())),
                           Gk[i0:i0 + k].min()))
            cols = [np.array(v) for v in zip(*mg)]
            keep2 = cols[8] * 4.0 * max(ylo, 1e-9) / sn2 < WSKIP
            tile_terms.append((cols, keep2))

        c1 = sum(len(tt[0][0]) for tt in tile_terms if tt)
        c2 = sum(int(tt[1].sum()) for tt in tile_terms if tt)
        tiles.append({"ylo": ylo, "yhi": yhi, "terms": tile_terms,
                      "c1": c1, "c2": c2})

    # process tiles in slot order = c1-ascending: batches of like-width
    # tiles (minimal padding), the narrow ones first so the pipeline
    # ramps while the DMA stream is still running and the wide batches
    # execute contention-free afterwards.  Host maps slots back later.
    slot_tiles = sorted(range(T), key=lambda t: tiles[t]["c1"])
    batches = []
    bstart = 0
    while bstart < T:
        bend = bstart + 1
        best = bstart + 1
        while bend <= T:
            sel = [tiles[slot_tiles[i]] for i in range(bstart, bend)]
            c1m = max(tl["c1"] for tl in sel)
            c2m = max(tl["c2"] for tl in sel)
            Bn = bend - bstart
            if Bn * (c1m + 3 + c2m) > PSUM_BUDGET or Bn > BMAX:
                break
            best = bend
            bend += 1
        bend = best
        sel = [tiles[slot_tiles[i]] for i in range(bstart, bend)]
        c1m = max(tl["c1"] for tl in sel)
        c2m = max(tl["c2"] for tl in sel)
        c1m += (c1m + 3) % 2
        batches.append((bstart, bend, c1m, c2m))
        bstart = bend

    return {"order_p": order_p, "mask_p": mask_p, "tiles": tiles,
            "batches": batches, "slot_tiles": slot_tiles,
            "logw": logw, "sn": sn, "c_u": c_u,
            "I": (I1, I2, I3)}


def _term_block(cols, sel, sgn):
    """rhs columns [ROWS, n] for one term's merged samples.  The ctb/cgb
    covariance corrections are folded into the t/g rows before the hi/lo
    split (they pair with the same x/y lhsT planes)."""
    tm, gm, Bm, vt, vg, ctg, ctb, cgb, _g = cols
    th, tl = _split(tm[sel] + ctb[sel])
    gh, gl = _split(sgn * (gm[sel] + cgb[sel]))
    bh, bl = _split(Bm[sel])
    n = len(th)
    one = np.ones(n, BF16)
    z = np.zeros(n, BF16)
    return np.stack([
        th, tl, th, gh, gl, gh, bh, bl,
        vt[sel].astype(BF16), vg[sel].astype(BF16),
        (sgn * ctg[sel]).astype(BF16),
        one, one, z, z,
    ]).astype(BF16)


def _pack(pl, x, y):
    """Build rhs [ROWS, NT] (shared), per-core lt [ROWS,T,P], mask, bvec."""
    sn = pl["sn"]; sn2 = sn * sn
    I1, I2, I3 = pl["I"]
    logw = pl["logw"]
    xs = np.asarray(x, np.float64)[pl["order_p"]]
    ysrt = np.asarray(y, np.float64)[pl["order_p"]]

    lny = np.log(ysrt)
    A = lny - 0.5 * (xs / sn) ** 2 - (ysrt / sn) ** 2
    Ps = [_interior_logp(xs, ysrt, I, sn) + logw[k]
          for k, I in enumerate((I1, I2, I3))]
    b_m = np.maximum.reduce(Ps + [lny + pl["c_u"]])
    nu64 = b_m - A
    nmh, nml = _split(-nu64)
    nu_use = -(nmh.astype(np.float64) + nml.astype(np.float64))
    bvec = (A + nu_use)
    lh, ll = _split(lny)

    ck = (math.log(2.0) - math.lgamma(1.5) - 4.0 * math.log(sn)
          - 0.5 * LOG2PI)
    intcols = np.zeros((ROWS, 3), BF16)
    for k, I in enumerate((I1, I2, I3)):
        tih, til = _split(np.array([I / sn2]))
        csth, cstl = _split(np.array([ck + logw[k] - 0.5 * I * I / sn2]))
        col = np.zeros(ROWS, BF16)
        col[0], col[1], col[2] = tih[0], til[0], tih[0]
        col[6], col[7] = csth[0], cstl[0]
        col[11] = col[12] = col[13] = col[14] = BF16(1.0)
        intcols[:, k] = col
    deadcol = np.zeros(ROWS, BF16)
    deadcol[6] = BF16(DEAD_B)

    # rhs: batch-major in SLOT order (slot s -> tile slot_tiles[s]),
    # per tile [C1' R1-cols | 3 interior | C2' R2-cols]
    slot_tiles = pl["slot_tiles"]
    rhs_parts = []
    meta = []
    off = 0
    for (t0, t1, C1, C2) in pl["batches"]:
        CW = C1 + 3 + C2
        for t in range(t0, t1):
            tt = pl["tiles"][slot_tiles[t]]
            blocks = []
            n1 = 0
            for j in range(3):
                ter = tt["terms"][j]
                if ter is None:
                    continue
                cols, keep2 = ter
                blk = _term_block(cols, slice(None), 1.0)
                blocks.append(blk)
                n1 += blk.shape[1]
            if n1 < C1:
                blocks.append(np.repeat(deadcol[:, None], C1 - n1, axis=1))
            blocks.append(intcols)
            n2 = 0
            for j in range(3):
                ter = tt["terms"][j]
                if ter is None:
                    continue
                cols, keep2 = ter
                if keep2.any():
                    blk = _term_block(cols, np.where(keep2)[0], -1.0)
                    blocks.append(blk)
                    n2 += blk.shape[1]
            if n2 < C2:
                blocks.append(np.repeat(deadcol[:, None], C2 - n2, axis=1))
            rhs_parts.append(np.concatenate(blocks, axis=1))
        meta.append((t0, t1, C1, C2, off))
        off += (t1 - t0) * CW
    rhs = np.concatenate(rhs_parts, axis=1).astype(BF16)
    NT = rhs.shape[1]
    assert NT == off

    xh, xl = _split(xs); yh, yl = _split(ysrt)
    planes = [
        xh, xh, xl, yh, yh, yl,
        np.ones(M_PAD, BF16), np.ones(M_PAD, BF16),
        (xs * xs).astype(BF16), (ysrt * ysrt).astype(BF16),
        (xs * ysrt).astype(BF16), nmh, nml, lh, ll,
    ]
    # pad slots: all-zero planes -> R = 0 for every column -> the slot
    # contributes exactly ln(C1'+3-C2') to the device sum (host-corrected)
    padm = pl["mask_p"] == 0.0
    planes = [np.where(padm, np.zeros(1, BF16), p).astype(BF16)
              for p in planes]

    # safety: packed R - nu must stay well under f32 exp overflow
    pf = np.stack([p.astype(np.float32) for p in planes]).astype(np.float64)
    rf = rhs.astype(np.float64)
    vmax = -1e30
    for (t0, t1, C1, C2, off0) in meta:
        CW = C1 + 3 + C2
        for s, t in enumerate(range(t0, t1)):
            tt = slot_tiles[t]
            blk = slice(tt * BLK, (tt + 1) * BLK)
            Rt = pf[:, blk].T @ rf[:, off0 + s * CW: off0 + (s + 1) * CW]
            vmax = max(vmax, float(Rt.max()))
    assert vmax < 60.0, f"exp overflow risk: max(R-nu)={vmax:.1f}"

    in_maps = []
    cores_bvec = []
    cores_mask = []
    st = np.asarray(slot_tiles)
    for i in range(N_CORES):
        lt = np.empty((ROWS, T, P), BF16)
        for r in range(ROWS):
            lt[r] = planes[r].reshape(T, N_CORES, P)[st, i, :]
        mask_i = np.ascontiguousarray(
            pl["mask_p"].reshape(T, N_CORES, P)[st, i, :].T).astype(
                np.float32)
        bvec_i = bvec.reshape(T, N_CORES, P)[st, i, :].T
        in_maps.append({"lt": lt, "rhs": rhs})
        cores_mask.append(mask_i)
        cores_bvec.append(bvec_i)
    return in_maps, meta, NT, cores_mask, cores_bvec


def _build(meta, NT):
    nc = bacc.Bacc("TRN2", target_bir_lowering=False, debug=False,
                   num_devices=N_CORES)
    dt_ = mybir.dt.float32
    bf = mybir.dt.bfloat16
    f = mybir.ActivationFunctionType
    alu = mybir.AluOpType

    lt_d = nc.dram_tensor("lt", [ROWS, T, P], bf, kind="ExternalInput").ap()
    rhs_d = nc.dram_tensor("rhs", [ROWS, NT], bf, kind="ExternalInput").ap()
    out_d = nc.dram_tensor("out", [P, T], dt_, kind="ExternalOutput").ap()

    def bank_slices(a, b):
        out = []
        while a < b:
            c = min(b, (a // 512 + 1) * 512)
            out.append((a, c))
            a = c
        return out

    with tile.TileContext(nc) as tc:
        with (
            tc.tile_pool(name="singles", bufs=1) as singles,
            tc.tile_pool(name="work", bufs=2) as work,
            tc.tile_pool(name="psum", bufs=2, space="PSUM") as psum_pool,
            tc.tile_pool(name="dump", bufs=2) as dump_pool,
        ):
            lt = singles.tile([ROWS, T, P], bf, tag="lt")
            rhs = singles.tile([ROWS, NT], bf, tag="rhs")
            # per-batch chunks, round-robin across the three DMA-capable
            # engines; batch 0's pair goes on the two HW-DGE queues so
            # the pipeline starts as soon as its chunks land
            engs = [nc.sync, nc.scalar, nc.gpsimd]
            qi = 2
            nb = len(meta)
            edges = [meta[0][0]] + [m[1] for m in meta]
            for ci in range(nb):
                a, b = edges[ci], edges[ci + 1]
                (t0, t1, C1, C2, off0) = meta[ci]
                w = (t1 - t0) * (C1 + 3 + C2)
                if ci == 0:
                    nc.sync.dma_start(lt[:, a:b, :], lt_d[:, a:b, :])
                    nc.scalar.dma_start(rhs[:, off0:off0 + w],
                                        rhs_d[:, off0:off0 + w])
                    continue
                engs[qi % 3].dma_start(lt[:, a:b, :], lt_d[:, a:b, :])
                qi += 1
                engs[qi % 3].dma_start(rhs[:, off0:off0 + w],
                                       rhs_d[:, off0:off0 + w])
                qi += 1

            S1 = singles.tile([P, T], dt_, tag="S1")
            S2 = singles.tile([P, T], dt_, tag="S2")
            nc.vector.memset(S2[:], 0.0)
            zb = singles.tile([P, 1], dt_, tag="zb")
            nc.vector.memset(zb[:], 0.0)
            sd = singles.tile([P, T], dt_, tag="sd")

            for (t0, t1, C1, C2, off0) in meta:
                CW = C1 + 3 + C2
                Bn = t1 - t0
                Wb = Bn * CW
                ps = psum_pool.tile([P, Wb], dt_, tag="ps", name="ps")
                dp = dump_pool.tile([P, Wb], dt_, tag="dp", name="dp")
                for s in range(Bn):
                    for a, b in bank_slices(s * CW, (s + 1) * CW):
                        nc.tensor.matmul(ps[:, a:b], lt[:, t0 + s, :],
                                         rhs[:, off0 + a:off0 + b],
                                         start=True, stop=True)
                nc.scalar.activation(dp[:], ps[:], f.Exp, bias=zb[:])
                r3 = dp.rearrange("p (b c) -> p b c", c=CW)
                nc.vector.tensor_reduce(S1[:, t0:t1], r3[:, :, 0:C1 + 3],
                                        mybir.AxisListType.X, alu.add)
                if C2 > 0:
                    nc.vector.tensor_reduce(S2[:, t0:t1],
                                            r3[:, :, C1 + 3:CW],
                                            mybir.AxisListType.X, alu.add)

            nc.vector.scalar_tensor_tensor(sd[:], S2[:], -1.0, S1[:],
                                           alu.mult, alu.add)
            nc.sync.dma_start(out_d, sd[:])

    nc.compile()
    return nc


def kernel(x, y, ku12, ku23, ku13, sigma_b, sigma_n, I1, I2, I3, w):
    pl = _plan(x, y, ku12, ku23, ku13, sigma_b, sigma_n, I1, I2, I3, w)
    in_maps, meta, NT, cores_mask, cores_bvec = _pack(pl, x, y)

    key = (NT, tuple((m[0], m[1], m[2], m[3]) for m in meta))
    if key not in _graph_cache:
        _graph_cache[key] = _build(meta, NT)
    nc = _graph_cache[key]

    res = run_bass_kernel_spmd(nc, in_maps, core_ids=list(range(N_CORES)))
    global _last_results
    _last_results = res

    loss = 0.0
    for i in range(N_CORES):
        sd = np.asarray(res.results[i]["out"], np.float64)
        lm = np.log(np.maximum(sd, 1e-300)) + cores_bvec[i]
        loss += float((lm * cores_mask[i]).sum())
    return np.float32(-loss)
